# revision 39
# baseline (speedup 1.0000x reference)
"""CIN (Compressed Interaction Network) kernel for Trainium2, 8 NeuronCores.

Reference computation (per layer k, fused einsum):
    xk_new[b,k,d] = sum_{i,j} W[k, i*n+j] * xk[b,i,d] * x0[b,j,d]
    pooled_k[b,:] = sum_d xk_new[b,:,d]
    out = concat(pooled_1, pooled_2, pooled_3)    # (B, 384)

Mapping:
  - Data-parallel over batch: 8 cores x 128 batches each.
  - On-chip layout: partitions = feature index i (H_prev), free dim =
    columns c = (b_local, d) pairs, processed in chunks of C columns.
  - Layer 1 exploits product symmetry x0_i*x0_j = x0_j*x0_i: host folds
    W into 528 unique pair rows (4 full 128-strips + one 16-strip) and
    precomputes the products, so L1 is 10 matmuls per chunk, no DVE.
  - Layer 2: loop j in 0..31:
        Y_j = x1 (.) broadcast(x0[:, j, :])    (VectorE tensor_tensor, bf16)
        psum[k, c] += W_j^T @ Y_j              (TensorE, K=H_prev contraction)
  - Layer 3 only needs the d-pooled output, so it avoids the 32 DVE
    products entirely: per 4-batch group g, PE transposes x2 into
    blocks x2T[(b4,d), i], computes S[i, (b4,j)] = x2T^T @ R_g with a
    host-built block-diagonal x0 matrix R_g[(b4,d),(b4',j)] (zero off
    the b4==b4' blocks), then contracts
        pooled3[b, k] = sum_j S_j^T @ W2_j     (S stationary, 32 matmuls)
  - pooled2[b, k] comes from block-ones matmuls against the same x2T
    blocks (free d-sum + transpose), pooled1 via a DVE d-group reduce.
  - All DRAM tensors are laid out so each DMA descriptor covers the
    full per-partition run (2-64 KB contiguous), and startup loads are
    ordered by first use (w0, w1, yl1, fja; w2/ones/srhs deferred).
"""

import os
import sys
from contextlib import ExitStack

sys.path.insert(0, "/opt/trn_rl_repo")
os.environ.setdefault("MYCRO_LOCAL_CACHE", "1")

import numpy as np
import ml_dtypes

import concourse.bass as bass
import concourse.tile as tile
from concourse import bacc, mybir
from concourse.bass_utils import run_bass_kernel_spmd
from concourse.masks import make_identity

B, N, D = 1024, 32, 32
H = 128                     # every layer's output features
NCORES = 8
BC = B // NCORES            # 128 batches per core
COLS = BC * D               # 4096 columns per core
C = 1024                    # chunk columns (32 batches x 32 d)
NB = C // D                 # batches per chunk
NCHUNK = COLS // C
MMN = 512                   # matmul moving free dim (one PSUM bank of fp32)
NPAIR = (N * (N + 1)) // 2  # 528 unique symmetric product pairs
NS4 = NPAIR - 4 * 128       # 16 rows in the last (partial) L1 strip
BF = mybir.dt.bfloat16
F32 = mybir.dt.float32

_CACHE = {}


def _dap(handle, offset, dims):
    a = handle[:]
    return bass.AP(tensor=a.tensor, offset=offset, ap=dims)


def _build_program():
    nc = bacc.Bacc(
        "TRN2", target_bir_lowering=False, debug=False, num_devices=NCORES
    )
    # per-partition-contiguous layouts (one DMA descriptor per partition)
    yfa = nc.declare_dram_parameter("yfa", [128, NCHUNK, 4, C], BF, isOutput=False)
    yfb = nc.declare_dram_parameter("yfb", [NS4, NCHUNK, C], BF, isOutput=False)
    fja = nc.declare_dram_parameter("fja", [NCHUNK, 128, N, C], BF, isOutput=False)
    w0pa = nc.declare_dram_parameter("w0pa", [128, 4, H], BF, isOutput=False)
    w0pb = nc.declare_dram_parameter("w0pb", [NS4, H], BF, isOutput=False)
    w1p = nc.declare_dram_parameter("w1p", [128, N, H], BF, isOutput=False)
    w2p = nc.declare_dram_parameter("w2p", [128, N, H], BF, isOutput=False)
    srhsp = nc.declare_dram_parameter("srhsp", [NCHUNK, 128, 8 * H], BF, isOutput=False)
    onesp = nc.declare_dram_parameter("onesp", [128, 8, NB], BF, isOutput=False)
    out = nc.declare_dram_parameter("out", [BC, 3 * H], F32, isOutput=True)

    with tile.TileContext(nc) as tc, ExitStack() as ctx:
        singles = ctx.enter_context(tc.tile_pool(name="singles", bufs=1))
        f4pool = ctx.enter_context(tc.tile_pool(name="f4pool", bufs=1))
        fpool = ctx.enter_context(tc.tile_pool(name="fpool", bufs=2))
        fpool1 = ctx.enter_context(tc.tile_pool(name="fpool1", bufs=1))
        xpool = ctx.enter_context(tc.tile_pool(name="xpool", bufs=3))
        ypool = ctx.enter_context(tc.tile_pool(name="ypool", bufs=4))
        xTpool = ctx.enter_context(tc.tile_pool(name="xTpool", bufs=2))
        srpool = ctx.enter_context(tc.tile_pool(name="srpool", bufs=4))
        pspool = ctx.enter_context(tc.tile_pool(name="ps", bufs=2, space="PSUM"))
        psTpool = ctx.enter_context(tc.tile_pool(name="psT", bufs=1, space="PSUM"))
        psSpool = ctx.enter_context(tc.tile_pool(name="psS", bufs=1, space="PSUM"))
        p2pool = ctx.enter_context(tc.tile_pool(name="p2", bufs=1, space="PSUM"))
        p3pool = ctx.enter_context(tc.tile_pool(name="p3", bufs=1, space="PSUM"))

        # --- startup loads, ordered by first use ---
        w0at = singles.tile([128, 4, H], BF)
        nc.sync.dma_start(out=w0at[:], in_=_dap(w0pa, 0, [[4 * H, 128], [H, 4], [1, H]]))
        w0bt = singles.tile([NS4, H], BF)
        nc.sync.dma_start(out=w0bt[:], in_=_dap(w0pb, 0, [[H, NS4], [1, H]]))
        w1t = singles.tile([128, N, H], BF)
        nc.scalar.dma_start(
            out=w1t[:], in_=_dap(w1p, 0, [[N * H, 128], [H, N], [1, H]])
        )
        w2t = singles.tile([128, N, H], BF)
        onest = singles.tile([128, 8, NB], BF)
        ident = singles.tile([128, 128], F32)
        make_identity(nc, ident[:])
        identb = singles.tile([128, 128], BF)
        nc.vector.tensor_copy(identb[:], ident[:])
        pooled1 = singles.tile([128, BC], F32)
        s_all = singles.tile([128, NCHUNK, 8 * H], BF)
        out_sb = singles.tile([128, 3 * H], F32)

        def bcast4(tile_ap):
            # (128, C) tile read as (128, 4, C) with the j-dim broadcast
            return bass.AP(
                tensor=tile_ap.tensor,
                offset=tile_ap.offset,
                ap=[tile_ap.ap[0], [0, 4], tile_ap.ap[1]],
            )

        NQ = N // 4  # j's per fj quarter tile

        def load_yl1(ich):
            # folded layer-1 products: 4 full strips (split in two loads so
            # the first strip matmuls can start earlier) + one 16-row strip
            ta0 = f4pool.tile([128, 2, C], BF, tag="f4a0", name=f"yl1a0_{ich}")
            nc.scalar.dma_start(
                out=ta0[:],
                in_=_dap(yfa, ich * 4 * C, [[NCHUNK * 4 * C, 128], [C, 2], [1, C]]),
            )
            ta1 = f4pool.tile([128, 2, C], BF, tag="f4a1", name=f"yl1a1_{ich}")
            nc.scalar.dma_start(
                out=ta1[:],
                in_=_dap(
                    yfa, ich * 4 * C + 2 * C, [[NCHUNK * 4 * C, 128], [C, 2], [1, C]]
                ),
            )
            tb = f4pool.tile([NS4, C], BF, tag="f4b", name=f"yl1b_{ich}")
            nc.scalar.dma_start(
                out=tb[:], in_=_dap(yfb, ich * C, [[NCHUNK * C, NS4], [1, C]])
            )
            return ta0, ta1, tb

        def load_fj(ich, quarters):
            # quarter tiles on alternating queues; quarters 0-2 are
            # double-buffered and prefetched at g==0, quarter 3 is
            # single-buffered and issued at g==5, keeping the DMA stream
            # fed instead of draining between chunk prefetch bursts
            tiles = []
            for qt in quarters:
                pool = fpool if qt < 3 else fpool1
                t = pool.tile([128, NQ, C], BF, tag=f"fjq{qt}", name=f"fj{ich}_{qt}")
                eng = nc.sync if qt % 2 == 0 else nc.scalar
                if qt == 0:
                    # split quarter 0 in two so the first quad's rows land
                    # in half the time (it gates the DVE at chunk start)
                    for hh in range(2):
                        eng.dma_start(
                            out=t[:, 4 * hh : 4 * (hh + 1), :],
                            in_=_dap(
                                fja,
                                ich * 128 * N * C + 4 * hh * C,
                                [[N * C, 128], [C, NQ // 2], [1, C]],
                            ),
                        )
                else:
                    eng.dma_start(
                        out=t[:],
                        in_=_dap(
                            fja,
                            ich * 128 * N * C + qt * NQ * C,
                            [[N * C, 128], [C, NQ], [1, C]],
                        ),
                    )
                tiles.append(t)
            return tiles

        def load_sr(ich):
            t = srpool.tile([128, 8, H], BF, tag="sr", name=f"sr{ich}")
            nc.sync.dma_start(
                out=t[:],
                in_=_dap(srhsp, ich * 128 * 8 * H, [[8 * H, 128], [H, 8], [1, H]]),
            )
            return t

        def layer1(yl1):
            ya0, ya1, yb = yl1
            ps1 = pspool.tile([128, C], F32, tag="ps")
            for s in range(4):
                ya = ya0 if s < 2 else ya1
                for t in range(C // MMN):
                    nc.tensor.matmul(
                        ps1[:, MMN * t : MMN * (t + 1)],
                        lhsT=w0at[:, s, :],
                        rhs=ya[:, s % 2, MMN * t : MMN * (t + 1)],
                        start=(s == 0),
                        stop=False,
                    )
            for t in range(C // MMN):
                nc.tensor.matmul(
                    ps1[:, MMN * t : MMN * (t + 1)],
                    lhsT=w0bt[:],
                    rhs=yb[:, MMN * t : MMN * (t + 1)],
                    start=False,
                    stop=True,
                )
            x1 = xpool.tile([128, C], BF, tag="x")
            nc.scalar.copy(out=x1[:], in_=ps1[:])
            return ps1, x1

        def reduce_ps1(ps, ich):
            nc.vector.reduce_sum(
                out=pooled1[:, ich * NB : (ich + 1) * NB],
                in_=ps[:].rearrange("p (b d) -> p b d", d=D),
                axis=mybir.AxisListType.X,
            )

        def quad(xk, wt, ps, fjt, g):
            j0 = 4 * g
            fh, fo = fjt[j0 // NQ], j0 % NQ
            y = ypool.tile([128, 4, C], BF, tag="y")
            nc.vector.tensor_mul(y[:], bcast4(xk[:]), fh[:, fo : fo + 4, :])
            for jl in range(4):
                j = j0 + jl
                for t in range(C // MMN):
                    nc.tensor.matmul(
                        ps[:, MMN * t : MMN * (t + 1)],
                        lhsT=wt[:, j, :],
                        rhs=y[:, jl, MMN * t : MMN * (t + 1)],
                        start=(j == 0),
                        stop=(j == N - 1),
                    )

        def l3_contract(ps3, ch0):
            # pooled3[b, k] over chunks {ch0, ch0+1} -> out partitions 64*h
            a = s_all[:]
            for j in range(N):
                s_j = bass.AP(
                    tensor=a.tensor,
                    offset=a.offset + ch0 * 8 * H + j,
                    ap=[a.ap[0], [8 * H, 2], [H, 8], [N, 4]],
                )
                nc.tensor.matmul(
                    ps3[64 * (ch0 // 2) : 64 * (ch0 // 2 + 1), :],
                    lhsT=s_j,
                    rhs=w2t[:, j, :],
                    start=(j == 0),
                    stop=(j == N - 1),
                )

        yl1n = load_yl1(0)
        fj = {0: load_fj(0, (0, 1, 2, 3))}
        ps1_next, x1_next = layer1(yl1n)
        rds = [(ps1_next, 0)]
        x1 = {0: x1_next}
        sr = {}
        ps3 = p3pool.tile([128, H], F32, tag="p3")

        def s_transposes(kk):
            # transpose x2(kk) into 4-batch blocks [(b4, d), i]
            psT = psTpool.tile([128, 8, H], BF, tag="psT", name=f"psT_{kk}")
            for g in range(8):
                nc.tensor.transpose(
                    psT[:, g, :], x2s[kk][:, 128 * g : 128 * (g + 1)], identb[:]
                )
            x2T = xTpool.tile([128, 8, H], BF, tag="xT", name=f"x2T_{kk}")
            nc.scalar.copy(out=x2T[:, 0:4, :], in_=psT[:, 0:4, :])
            nc.scalar.copy(out=x2T[:, 4:8, :], in_=psT[:, 4:8, :])
            return x2T

        def s_half(kk, x2T, half):
            # S[i, (b4', j)] for 4 groups, via block-diagonal x0 rhs
            psS = psSpool.tile([128, 4, H], F32, tag="psS", name=f"psS_{kk}_{half}")
            for gl in range(4):
                g = 4 * half + gl
                nc.tensor.matmul(
                    psS[:, gl, :],
                    lhsT=x2T[:, g, :],
                    rhs=sr[kk][:, g, :],
                    start=True,
                    stop=True,
                )
            nc.scalar.copy(
                out=s_all[:, kk, 4 * H * half : 4 * H * (half + 1)],
                in_=psS[:].rearrange("p g h -> p (g h)"),
            )

        def p2_block(kk, x2T):
            # pooled2 strip: block-ones d-sums of x2T, already (b, k) order
            p2 = p2pool.tile([32, H], F32, tag="p2", name=f"p2_{kk}")
            for g in range(8):
                nc.tensor.matmul(
                    p2[:],
                    lhsT=onest[:, g, :],
                    rhs=x2T[:, g, :],
                    start=(g == 0),
                    stop=(g == 7),
                )
            nc.scalar.copy(out=out_sb[32 * kk : 32 * (kk + 1), H : 2 * H], in_=p2[:])

        # Software pipeline: chunk k's L2 quad loop hides (a) chunk k-1's
        # whole layer-3 S block (transposes at g1, S halves at g2/g5, so
        # each PE stage's Activation-copy input is ready well in advance)
        # and (b) chunk k+1's layer 1 at g4 (so x1 never gates the DVE).
        x2s = {}
        x2T_prev = None
        for k in range(NCHUNK):
            # pooled1 reduces for psums whose groups closed long ago
            for ps_, ich_ in rds:
                reduce_ps1(ps_, ich_)
            rds.clear()
            ps2 = pspool.tile([128, C], F32, tag="ps", name=f"ps2_{k}")
            for g in range(8):
                quad(x1[k], w1t, ps2, fj[k], g)
                if g == 0:
                    if k + 1 < NCHUNK:
                        yl1n = load_yl1(k + 1)
                        fj[k + 1] = load_fj(k + 1, (0, 1, 2))
                        sr[k + 1] = load_sr(k + 1)
                    if k == 0:
                        # deferred startup loads: first needed at chunk-0 end
                        sr[0] = load_sr(0)
                        nc.sync.dma_start(
                            out=onest[:],
                            in_=_dap(onesp, 0, [[8 * NB, 128], [NB, 8], [1, NB]]),
                        )
                        nc.scalar.dma_start(
                            out=w2t[:],
                            in_=_dap(w2p, 0, [[N * H, 128], [H, N], [1, H]]),
                        )
                if g == 4 and k + 1 < NCHUNK:
                    # layer 1 of chunk k+1, interleaved mid-L2 so its x1
                    # copy never gates the DVE at the chunk boundary
                    ps1_next, x1[k + 1] = layer1(yl1n)
                    rds.append((ps1_next, k + 1))
                if g == 5 and k + 1 < NCHUNK:
                    fj[k + 1].extend(load_fj(k + 1, (3,)))
            x2s[k] = xpool.tile([128, C], BF, tag="x", name=f"x2_{k}")
            nc.scalar.copy(out=x2s[k][:], in_=ps2[:])
            # ---- layer 3 of chunk k, S-form ----
            x2T = s_transposes(k)
            s_half(k, x2T, 0)
            s_half(k, x2T, 1)
            p2_block(k, x2T)
            if k == 1:
                l3_contract(ps3, 0)
        for ps_, ich_ in rds:
            reduce_ps1(ps_, ich_)
        l3_contract(ps3, 2)

        # ---- finalize: pooled1 (k, b) -> (b, k) via PE; others are (b, k) ----
        tp = psSpool.tile([128, 128], F32, tag="psS", name="tp1")
        nc.tensor.transpose(tp[:], pooled1[:], ident[:])
        nc.scalar.copy(out=out_sb[:, 0:H], in_=tp[:])
        nc.scalar.copy(out=out_sb[:, 2 * H : 3 * H], in_=ps3[:])
        nc.sync.dma_start(out=out[:], in_=out_sb[:])

    nc.compile()
    return nc


def _fold_pairs():
    # 528 unique (i <= j) pairs: 32 diagonal first, then upper triangle
    pairs = [(i, i) for i in range(N)]
    pairs += [(i, j) for i in range(N) for j in range(i + 1, N)]
    return np.array(pairs, np.int64)  # (528, 2)


def _prep_inputs(x0, w0, w1, w2):
    bf = ml_dtypes.bfloat16
    x0b = np.ascontiguousarray(x0.astype(bf))
    pairs = _fold_pairs()
    # folded w0: wf[p] = w0[i*N+j] + w0[j*N+i] (i < j), w0[i*N+i] on diag
    w3 = w0.reshape(N, N, H)
    ii, jj = pairs[:, 0], pairs[:, 1]
    wf = w3[ii, jj] + np.where((ii != jj)[:, None], w3[jj, ii], 0.0)  # (528, H)
    w0qa = np.ascontiguousarray(
        wf[: 4 * 128].reshape(4, 128, H).transpose(1, 0, 2).astype(bf)
    )  # [128, 4, H]
    w0qb = np.ascontiguousarray(wf[4 * 128 :].astype(bf))  # [16, H]
    w1r = np.ascontiguousarray(
        w1.reshape(H, N, H).transpose(1, 0, 2).astype(bf)   # (j, i, k)
    )
    w2r = np.ascontiguousarray(
        w2.reshape(H, N, H).transpose(1, 0, 2).astype(bf)
    )
    # relayout (j, i, k) -> (i, j, k) for contiguous per-partition loads
    w1q = np.ascontiguousarray(w1r.transpose(1, 0, 2))
    w2q = np.ascontiguousarray(w2r.transpose(1, 0, 2))
    # block-ones patterns for pooled2: ones_g[(b4 d), b'] = [b' == 4g + b4]
    ones = np.zeros((8, 4, D, NB), np.float32)
    for g in range(8):
        for b4 in range(4):
            ones[g, b4, :, 4 * g + b4] = 1.0
    onesq = np.ascontiguousarray(
        ones.reshape(8, 128, NB).transpose(1, 0, 2).astype(bf)
    )  # [128, 8, NB]
    return x0b, w0qa, w0qb, w1q, w2q, onesq, pairs


def _get_compiled():
    if "nc" not in _CACHE:
        _CACHE["nc"] = _build_program()
    return _CACHE["nc"]


def run(x0, w0, w1, w2, trace=False):
    nc = _get_compiled()
    x0b, w0qa, w0qb, w1q, w2q, onesq, pairs = _prep_inputs(
        np.asarray(x0, np.float32),
        np.asarray(w0, np.float32),
        np.asarray(w1, np.float32),
        np.asarray(w2, np.float32),
    )
    bf = ml_dtypes.bfloat16
    core_ids = list(range(NCORES))
    ii, jj = pairs[:, 0], pairs[:, 1]
    in_maps = []
    for c in core_ids:
        shard = np.ascontiguousarray(x0b[c * BC : (c + 1) * BC])
        x0t = np.ascontiguousarray(shard.transpose(1, 0, 2).reshape(N, COLS))
        x0f = x0t.astype(np.float32)
        # folded layer-1 products, strip-major then per-partition chunked
        yl1f = (x0f[ii] * x0f[jj]).astype(bf)                 # (528, COLS)
        yfa_np = np.ascontiguousarray(
            yl1f[: 4 * 128].reshape(4, 128, NCHUNK, C).transpose(1, 2, 0, 3)
        )  # [128, NCHUNK, 4, C]
        yfb_np = np.ascontiguousarray(
            yl1f[4 * 128 :].reshape(NS4, NCHUNK, C)
        )
        # fja: j-broadcast, per-partition-contiguous per chunk
        fja_np = np.ascontiguousarray(
            np.broadcast_to(
                x0t.reshape(N, NCHUNK, C)[None], (128, N, NCHUNK, C)
            ).transpose(2, 0, 1, 3)
        )  # [NCHUNK, 128, N, C]
        # block-diagonal x0 rhs blocks for the layer-3 S matmuls
        xT = shard.astype(np.float32).transpose(0, 2, 1).reshape(NCHUNK, NB, D, N)
        srhs = np.zeros((NCHUNK, 4, D, 8, 4, N), np.float32)
        for b4 in range(4):
            srhs[:, b4, :, :, b4, :] = xT[:, 4 * np.arange(8) + b4].transpose(
                0, 2, 1, 3
            )
        in_maps.append(
            {
                "yfa": yfa_np,
                "yfb": yfb_np,
                "fja": fja_np,
                "w0pa": w0qa,
                "w0pb": w0qb,
                "w1p": w1q,
                "w2p": w2q,
                "srhsp": np.ascontiguousarray(
                    srhs.reshape(NCHUNK, 128, 8 * H).astype(bf)
                ),
                "onesp": onesq,
            }
        )
    res = run_bass_kernel_spmd(nc, in_maps, core_ids, trace=trace)
    outs = [np.asarray(res.results[c]["out"], np.float32) for c in core_ids]
    return np.concatenate(outs, axis=0), res


def kernel(x0, w0, w1, w2):
    full, _ = run(x0, w0, w1, w2, trace=False)
    return full


# revision 40
# speedup vs baseline: 1.0011x; 1.0011x over previous
"""CIN (Compressed Interaction Network) kernel for Trainium2, 8 NeuronCores.

Reference computation (per layer k, fused einsum):
    xk_new[b,k,d] = sum_{i,j} W[k, i*n+j] * xk[b,i,d] * x0[b,j,d]
    pooled_k[b,:] = sum_d xk_new[b,:,d]
    out = concat(pooled_1, pooled_2, pooled_3)    # (B, 384)

Mapping:
  - Data-parallel over batch: 8 cores x 128 batches each.
  - On-chip layout: partitions = feature index i (H_prev), free dim =
    columns c = (b_local, d) pairs, processed in chunks of C columns.
  - Layer 1 exploits product symmetry x0_i*x0_j = x0_j*x0_i: host folds
    W into 528 unique pair rows (4 full 128-strips + one 16-strip) and
    precomputes the products, so L1 is 10 matmuls per chunk, no DVE.
  - Layer 2: loop j in 0..31:
        Y_j = x1 (.) broadcast(x0[:, j, :])    (VectorE tensor_tensor, bf16)
        psum[k, c] += W_j^T @ Y_j              (TensorE, K=H_prev contraction)
  - Layer 3 only needs the d-pooled output, so it avoids the 32 DVE
    products entirely: per 4-batch group g, PE transposes x2 into
    blocks x2T[(b4,d), i], computes S[i, (b4,j)] = x2T^T @ R_g with a
    host-built block-diagonal x0 matrix R_g[(b4,d),(b4',j)] (zero off
    the b4==b4' blocks), then contracts
        pooled3[b, k] = sum_j S_j^T @ W2_j     (S stationary, 32 matmuls)
  - pooled2[b, k] comes from block-ones matmuls against the same x2T
    blocks (free d-sum + transpose), pooled1 via a DVE d-group reduce.
  - All DRAM tensors are laid out so each DMA descriptor covers the
    full per-partition run (2-64 KB contiguous), and startup loads are
    ordered by first use (w0, w1, yl1, fja; w2/ones/srhs deferred).
"""

import os
import sys
from contextlib import ExitStack

sys.path.insert(0, "/opt/trn_rl_repo")
os.environ.setdefault("MYCRO_LOCAL_CACHE", "1")

import numpy as np
import ml_dtypes

import concourse.bass as bass
import concourse.tile as tile
from concourse import bacc, mybir
from concourse.bass_utils import run_bass_kernel_spmd
from concourse.masks import make_identity

B, N, D = 1024, 32, 32
H = 128                     # every layer's output features
NCORES = 8
BC = B // NCORES            # 128 batches per core
COLS = BC * D               # 4096 columns per core
C = 1024                    # chunk columns (32 batches x 32 d)
NB = C // D                 # batches per chunk
NCHUNK = COLS // C
MMN = 512                   # matmul moving free dim (one PSUM bank of fp32)
NPAIR = (N * (N + 1)) // 2  # 528 unique symmetric product pairs
NS4 = NPAIR - 4 * 128       # 16 rows in the last (partial) L1 strip
BF = mybir.dt.bfloat16
F32 = mybir.dt.float32

_CACHE = {}


def _dap(handle, offset, dims):
    a = handle[:]
    return bass.AP(tensor=a.tensor, offset=offset, ap=dims)


def _build_program():
    nc = bacc.Bacc(
        "TRN2", target_bir_lowering=False, debug=False, num_devices=NCORES
    )
    # per-partition-contiguous layouts (one DMA descriptor per partition)
    yfa = nc.declare_dram_parameter("yfa", [128, NCHUNK, 4, C], BF, isOutput=False)
    yfb = nc.declare_dram_parameter("yfb", [NS4, NCHUNK, C], BF, isOutput=False)
    fja = nc.declare_dram_parameter("fja", [NCHUNK, 128, N, C], BF, isOutput=False)
    w0pa = nc.declare_dram_parameter("w0pa", [128, 4, H], BF, isOutput=False)
    w0pb = nc.declare_dram_parameter("w0pb", [NS4, H], BF, isOutput=False)
    w1p = nc.declare_dram_parameter("w1p", [128, N, H], BF, isOutput=False)
    w2p = nc.declare_dram_parameter("w2p", [128, N, H], BF, isOutput=False)
    srhsp = nc.declare_dram_parameter("srhsp", [NCHUNK, 128, 8 * H], BF, isOutput=False)
    onesp = nc.declare_dram_parameter("onesp", [128, 8, NB], BF, isOutput=False)
    out = nc.declare_dram_parameter("out", [BC, 3 * H], F32, isOutput=True)

    with tile.TileContext(nc) as tc, ExitStack() as ctx:
        singles = ctx.enter_context(tc.tile_pool(name="singles", bufs=1))
        f4pool = ctx.enter_context(tc.tile_pool(name="f4pool", bufs=1))
        fpool = ctx.enter_context(tc.tile_pool(name="fpool", bufs=2))
        fpool1 = ctx.enter_context(tc.tile_pool(name="fpool1", bufs=1))
        xpool = ctx.enter_context(tc.tile_pool(name="xpool", bufs=3))
        ypool = ctx.enter_context(tc.tile_pool(name="ypool", bufs=4))
        xTpool = ctx.enter_context(tc.tile_pool(name="xTpool", bufs=2))
        srpool = ctx.enter_context(tc.tile_pool(name="srpool", bufs=4))
        pspool = ctx.enter_context(tc.tile_pool(name="ps", bufs=2, space="PSUM"))
        psTpool = ctx.enter_context(tc.tile_pool(name="psT", bufs=1, space="PSUM"))
        psSpool = ctx.enter_context(tc.tile_pool(name="psS", bufs=1, space="PSUM"))
        p2pool = ctx.enter_context(tc.tile_pool(name="p2", bufs=1, space="PSUM"))
        p3pool = ctx.enter_context(tc.tile_pool(name="p3", bufs=1, space="PSUM"))

        # --- startup loads, ordered by first use ---
        w0at = singles.tile([128, 4, H], BF)
        nc.sync.dma_start(out=w0at[:], in_=_dap(w0pa, 0, [[4 * H, 128], [H, 4], [1, H]]))
        w0bt = singles.tile([NS4, H], BF)
        nc.sync.dma_start(out=w0bt[:], in_=_dap(w0pb, 0, [[H, NS4], [1, H]]))
        w1t = singles.tile([128, N, H], BF)
        nc.scalar.dma_start(
            out=w1t[:], in_=_dap(w1p, 0, [[N * H, 128], [H, N], [1, H]])
        )
        w2t = singles.tile([128, N, H], BF)
        onest = singles.tile([128, 8, NB], BF)
        ident = singles.tile([128, 128], F32)
        make_identity(nc, ident[:])
        identb = singles.tile([128, 128], BF)
        nc.vector.tensor_copy(identb[:], ident[:])
        pooled1 = singles.tile([128, BC], F32)
        s_all = singles.tile([128, NCHUNK, 8 * H], BF)
        out_sb = singles.tile([128, 3 * H], F32)

        def bcast4(tile_ap):
            # (128, C) tile read as (128, 4, C) with the j-dim broadcast
            return bass.AP(
                tensor=tile_ap.tensor,
                offset=tile_ap.offset,
                ap=[tile_ap.ap[0], [0, 4], tile_ap.ap[1]],
            )

        NQ = N // 4  # j's per fj quarter tile

        def load_yl1(ich):
            # folded layer-1 products: 4 full strips (split in two loads so
            # the first strip matmuls can start earlier) + one 16-row strip
            ta0 = f4pool.tile([128, 2, C], BF, tag="f4a0", name=f"yl1a0_{ich}")
            nc.scalar.dma_start(
                out=ta0[:],
                in_=_dap(yfa, ich * 4 * C, [[NCHUNK * 4 * C, 128], [C, 2], [1, C]]),
            )
            ta1 = f4pool.tile([128, 2, C], BF, tag="f4a1", name=f"yl1a1_{ich}")
            nc.scalar.dma_start(
                out=ta1[:],
                in_=_dap(
                    yfa, ich * 4 * C + 2 * C, [[NCHUNK * 4 * C, 128], [C, 2], [1, C]]
                ),
            )
            tb = f4pool.tile([NS4, C], BF, tag="f4b", name=f"yl1b_{ich}")
            nc.scalar.dma_start(
                out=tb[:], in_=_dap(yfb, ich * C, [[NCHUNK * C, NS4], [1, C]])
            )
            return ta0, ta1, tb

        def load_fj(ich, quarters):
            # quarter tiles on alternating queues; quarters 0-2 are
            # double-buffered and prefetched at g==0, quarter 3 is
            # single-buffered and issued at g==5, keeping the DMA stream
            # fed instead of draining between chunk prefetch bursts
            tiles = []
            for qt in quarters:
                pool = fpool if qt < 3 else fpool1
                t = pool.tile([128, NQ, C], BF, tag=f"fjq{qt}", name=f"fj{ich}_{qt}")
                eng = nc.sync if qt % 2 == 0 else nc.scalar
                eng.dma_start(
                    out=t[:],
                    in_=_dap(
                        fja,
                        ich * 128 * N * C + qt * NQ * C,
                        [[N * C, 128], [C, NQ], [1, C]],
                    ),
                )
                tiles.append(t)
            return tiles

        def load_sr(ich):
            t = srpool.tile([128, 8, H], BF, tag="sr", name=f"sr{ich}")
            nc.sync.dma_start(
                out=t[:],
                in_=_dap(srhsp, ich * 128 * 8 * H, [[8 * H, 128], [H, 8], [1, H]]),
            )
            return t

        def layer1(yl1):
            ya0, ya1, yb = yl1
            ps1 = pspool.tile([128, C], F32, tag="ps")
            for s in range(4):
                ya = ya0 if s < 2 else ya1
                for t in range(C // MMN):
                    nc.tensor.matmul(
                        ps1[:, MMN * t : MMN * (t + 1)],
                        lhsT=w0at[:, s, :],
                        rhs=ya[:, s % 2, MMN * t : MMN * (t + 1)],
                        start=(s == 0),
                        stop=False,
                    )
            for t in range(C // MMN):
                nc.tensor.matmul(
                    ps1[:, MMN * t : MMN * (t + 1)],
                    lhsT=w0bt[:],
                    rhs=yb[:, MMN * t : MMN * (t + 1)],
                    start=False,
                    stop=True,
                )
            x1 = xpool.tile([128, C], BF, tag="x")
            nc.scalar.copy(out=x1[:], in_=ps1[:])
            return ps1, x1

        def reduce_ps1(ps, ich):
            nc.vector.reduce_sum(
                out=pooled1[:, ich * NB : (ich + 1) * NB],
                in_=ps[:].rearrange("p (b d) -> p b d", d=D),
                axis=mybir.AxisListType.X,
            )

        def quad(xk, wt, ps, fjt, g):
            j0 = 4 * g
            fh, fo = fjt[j0 // NQ], j0 % NQ
            y = ypool.tile([128, 4, C], BF, tag="y")
            nc.vector.tensor_mul(y[:], bcast4(xk[:]), fh[:, fo : fo + 4, :])
            for jl in range(4):
                j = j0 + jl
                for t in range(C // MMN):
                    nc.tensor.matmul(
                        ps[:, MMN * t : MMN * (t + 1)],
                        lhsT=wt[:, j, :],
                        rhs=y[:, jl, MMN * t : MMN * (t + 1)],
                        start=(j == 0),
                        stop=(j == N - 1),
                    )

        def l3_contract(ps3, ch0):
            # pooled3[b, k] over chunks {ch0, ch0+1} -> out partitions 64*h
            a = s_all[:]
            for j in range(N):
                s_j = bass.AP(
                    tensor=a.tensor,
                    offset=a.offset + ch0 * 8 * H + j,
                    ap=[a.ap[0], [8 * H, 2], [H, 8], [N, 4]],
                )
                nc.tensor.matmul(
                    ps3[64 * (ch0 // 2) : 64 * (ch0 // 2 + 1), :],
                    lhsT=s_j,
                    rhs=w2t[:, j, :],
                    start=(j == 0),
                    stop=(j == N - 1),
                )

        yl1n = load_yl1(0)
        fj = {0: load_fj(0, (0, 1, 2, 3))}
        ps1_next, x1_next = layer1(yl1n)
        rds = [(ps1_next, 0)]
        x1 = {0: x1_next}
        sr = {}
        ps3 = p3pool.tile([128, H], F32, tag="p3")

        def s_transposes(kk):
            # transpose x2(kk) into 4-batch blocks [(b4, d), i]
            psT = psTpool.tile([128, 8, H], BF, tag="psT", name=f"psT_{kk}")
            for g in range(8):
                nc.tensor.transpose(
                    psT[:, g, :], x2s[kk][:, 128 * g : 128 * (g + 1)], identb[:]
                )
            x2T = xTpool.tile([128, 8, H], BF, tag="xT", name=f"x2T_{kk}")
            nc.scalar.copy(out=x2T[:, 0:4, :], in_=psT[:, 0:4, :])
            nc.scalar.copy(out=x2T[:, 4:8, :], in_=psT[:, 4:8, :])
            return x2T

        def s_half(kk, x2T, half):
            # S[i, (b4', j)] for 4 groups, via block-diagonal x0 rhs
            psS = psSpool.tile([128, 4, H], F32, tag="psS", name=f"psS_{kk}_{half}")
            for gl in range(4):
                g = 4 * half + gl
                nc.tensor.matmul(
                    psS[:, gl, :],
                    lhsT=x2T[:, g, :],
                    rhs=sr[kk][:, g, :],
                    start=True,
                    stop=True,
                )
            nc.scalar.copy(
                out=s_all[:, kk, 4 * H * half : 4 * H * (half + 1)],
                in_=psS[:].rearrange("p g h -> p (g h)"),
            )

        def p2_block(kk, x2T):
            # pooled2 strip: block-ones d-sums of x2T, already (b, k) order
            p2 = p2pool.tile([32, H], F32, tag="p2", name=f"p2_{kk}")
            for g in range(8):
                nc.tensor.matmul(
                    p2[:],
                    lhsT=onest[:, g, :],
                    rhs=x2T[:, g, :],
                    start=(g == 0),
                    stop=(g == 7),
                )
            nc.scalar.copy(out=out_sb[32 * kk : 32 * (kk + 1), H : 2 * H], in_=p2[:])

        # Software pipeline: chunk k's L2 quad loop hides (a) chunk k-1's
        # whole layer-3 S block (transposes at g1, S halves at g2/g5, so
        # each PE stage's Activation-copy input is ready well in advance)
        # and (b) chunk k+1's layer 1 at g4 (so x1 never gates the DVE).
        x2s = {}
        x2T_prev = None
        for k in range(NCHUNK):
            # pooled1 reduces for psums whose groups closed long ago
            for ps_, ich_ in rds:
                reduce_ps1(ps_, ich_)
            rds.clear()
            ps2 = pspool.tile([128, C], F32, tag="ps", name=f"ps2_{k}")
            for g in range(8):
                quad(x1[k], w1t, ps2, fj[k], g)
                if g == 0:
                    if k + 1 < NCHUNK:
                        yl1n = load_yl1(k + 1)
                        fj[k + 1] = load_fj(k + 1, (0, 1, 2))
                        sr[k + 1] = load_sr(k + 1)
                    if k == 0:
                        # deferred startup loads: first needed at chunk-0 end
                        sr[0] = load_sr(0)
                        nc.sync.dma_start(
                            out=onest[:],
                            in_=_dap(onesp, 0, [[8 * NB, 128], [NB, 8], [1, NB]]),
                        )
                        nc.scalar.dma_start(
                            out=w2t[:],
                            in_=_dap(w2p, 0, [[N * H, 128], [H, N], [1, H]]),
                        )
                if g == 4 and k + 1 < NCHUNK:
                    # layer 1 of chunk k+1, interleaved mid-L2 so its x1
                    # copy never gates the DVE at the chunk boundary
                    ps1_next, x1[k + 1] = layer1(yl1n)
                    rds.append((ps1_next, k + 1))
                if g == 5 and k + 1 < NCHUNK:
                    fj[k + 1].extend(load_fj(k + 1, (3,)))
            x2s[k] = xpool.tile([128, C], BF, tag="x", name=f"x2_{k}")
            nc.scalar.copy(out=x2s[k][:], in_=ps2[:])
            # ---- layer 3 of chunk k, S-form ----
            x2T = s_transposes(k)
            s_half(k, x2T, 0)
            s_half(k, x2T, 1)
            p2_block(k, x2T)
            if k == 1:
                l3_contract(ps3, 0)
        for ps_, ich_ in rds:
            reduce_ps1(ps_, ich_)
        l3_contract(ps3, 2)

        # ---- finalize: pooled1 (k, b) -> (b, k) via PE; others are (b, k) ----
        tp = psSpool.tile([128, 128], F32, tag="psS", name="tp1")
        nc.tensor.transpose(tp[:], pooled1[:], ident[:])
        nc.scalar.copy(out=out_sb[:, 0:H], in_=tp[:])
        nc.scalar.copy(out=out_sb[:, 2 * H : 3 * H], in_=ps3[:])
        nc.sync.dma_start(out=out[:], in_=out_sb[:])

    nc.compile()
    return nc


def _fold_pairs():
    # 528 unique (i <= j) pairs: 32 diagonal first, then upper triangle
    pairs = [(i, i) for i in range(N)]
    pairs += [(i, j) for i in range(N) for j in range(i + 1, N)]
    return np.array(pairs, np.int64)  # (528, 2)


def _prep_inputs(x0, w0, w1, w2):
    bf = ml_dtypes.bfloat16
    x0b = np.ascontiguousarray(x0.astype(bf))
    pairs = _fold_pairs()
    # folded w0: wf[p] = w0[i*N+j] + w0[j*N+i] (i < j), w0[i*N+i] on diag
    w3 = w0.reshape(N, N, H)
    ii, jj = pairs[:, 0], pairs[:, 1]
    wf = w3[ii, jj] + np.where((ii != jj)[:, None], w3[jj, ii], 0.0)  # (528, H)
    w0qa = np.ascontiguousarray(
        wf[: 4 * 128].reshape(4, 128, H).transpose(1, 0, 2).astype(bf)
    )  # [128, 4, H]
    w0qb = np.ascontiguousarray(wf[4 * 128 :].astype(bf))  # [16, H]
    w1r = np.ascontiguousarray(
        w1.reshape(H, N, H).transpose(1, 0, 2).astype(bf)   # (j, i, k)
    )
    w2r = np.ascontiguousarray(
        w2.reshape(H, N, H).transpose(1, 0, 2).astype(bf)
    )
    # relayout (j, i, k) -> (i, j, k) for contiguous per-partition loads
    w1q = np.ascontiguousarray(w1r.transpose(1, 0, 2))
    w2q = np.ascontiguousarray(w2r.transpose(1, 0, 2))
    # block-ones patterns for pooled2: ones_g[(b4 d), b'] = [b' == 4g + b4]
    ones = np.zeros((8, 4, D, NB), np.float32)
    for g in range(8):
        for b4 in range(4):
            ones[g, b4, :, 4 * g + b4] = 1.0
    onesq = np.ascontiguousarray(
        ones.reshape(8, 128, NB).transpose(1, 0, 2).astype(bf)
    )  # [128, 8, NB]
    return x0b, w0qa, w0qb, w1q, w2q, onesq, pairs


def _get_compiled():
    if "nc" not in _CACHE:
        _CACHE["nc"] = _build_program()
    return _CACHE["nc"]


def run(x0, w0, w1, w2, trace=False):
    nc = _get_compiled()
    x0b, w0qa, w0qb, w1q, w2q, onesq, pairs = _prep_inputs(
        np.asarray(x0, np.float32),
        np.asarray(w0, np.float32),
        np.asarray(w1, np.float32),
        np.asarray(w2, np.float32),
    )
    bf = ml_dtypes.bfloat16
    core_ids = list(range(NCORES))
    ii, jj = pairs[:, 0], pairs[:, 1]
    in_maps = []
    for c in core_ids:
        shard = np.ascontiguousarray(x0b[c * BC : (c + 1) * BC])
        x0t = np.ascontiguousarray(shard.transpose(1, 0, 2).reshape(N, COLS))
        x0f = x0t.astype(np.float32)
        # folded layer-1 products, strip-major then per-partition chunked
        yl1f = (x0f[ii] * x0f[jj]).astype(bf)                 # (528, COLS)
        yfa_np = np.ascontiguousarray(
            yl1f[: 4 * 128].reshape(4, 128, NCHUNK, C).transpose(1, 2, 0, 3)
        )  # [128, NCHUNK, 4, C]
        yfb_np = np.ascontiguousarray(
            yl1f[4 * 128 :].reshape(NS4, NCHUNK, C)
        )
        # fja: j-broadcast, per-partition-contiguous per chunk
        fja_np = np.ascontiguousarray(
            np.broadcast_to(
                x0t.reshape(N, NCHUNK, C)[None], (128, N, NCHUNK, C)
            ).transpose(2, 0, 1, 3)
        )  # [NCHUNK, 128, N, C]
        # block-diagonal x0 rhs blocks for the layer-3 S matmuls
        xT = shard.astype(np.float32).transpose(0, 2, 1).reshape(NCHUNK, NB, D, N)
        srhs = np.zeros((NCHUNK, 4, D, 8, 4, N), np.float32)
        for b4 in range(4):
            srhs[:, b4, :, :, b4, :] = xT[:, 4 * np.arange(8) + b4].transpose(
                0, 2, 1, 3
            )
        in_maps.append(
            {
                "yfa": yfa_np,
                "yfb": yfb_np,
                "fja": fja_np,
                "w0pa": w0qa,
                "w0pb": w0qb,
                "w1p": w1q,
                "w2p": w2q,
                "srhsp": np.ascontiguousarray(
                    srhs.reshape(NCHUNK, 128, 8 * H).astype(bf)
                ),
                "onesp": onesq,
            }
        )
    res = run_bass_kernel_spmd(nc, in_maps, core_ids, trace=trace)
    outs = [np.asarray(res.results[c]["out"], np.float32) for c in core_ids]
    return np.concatenate(outs, axis=0), res


def kernel(x0, w0, w1, w2):
    full, _ = run(x0, w0, w1, w2, trace=False)
    return full
